# revision 25
# baseline (speedup 1.0000x reference)
"""Trainium2 Bass kernel for nn_Affine_83811991814656 (MoE routing).

Math (reference):
    out[t] = sum_e gates[t, e] * (x[t] @ W[e].T + b[e])

Strategy:
  - Tokens (B*S = 16384) are data-parallel across 8 NeuronCores. SPMD: one
    program, per-core data, so the tile -> experts map must be identical on
    every core.
  - Gates are top-2-of-4 sparse. Host groups tokens by nonzero-expert set
    (6 pair groups typically). Each group fills whole 128-token-per-core
    tiles ("pure" tiles, 2 expert slots); the <1-tile remainders are
    bin-packed into ~3 shared spill tiles whose slots are the union of the
    packed groups (zero gates annihilate wrong-expert contributions), with
    the packing order brute-forced to minimize slot count. Zero padding in
    the typical case: 16 tiles, ~35 slots vs 64 for a dense kernel.
  - Per tile the gate scaling is folded into the matmul lhsT (xg = xT * g
    broadcast along partitions, on DVE), so all expert matmuls AND the
    bias matmul (gates^T @ b, zero-padded to K=128 - a K=4 fp32r matmul
    measures ~3x slower than K=128) accumulate into one PSUM bank per
    output half. fp32r matmuls: full fp32 storage, PE rounds to ~13
    mantissa bits (rel err ~1.5e-4), 4x faster than strict fp32, and the
    ~512-row stream dominates so per-matmul cost is ~240 ns.
  - W (all 4 experts, transposed to [d, o]) stays resident in SBUF (16 MB).
  - DMA spread over the three queues: W/gbc/gates on the SWDGE (gpsimd)
    queue, x loads on the SP HWDGE ring, psum->SBUF copies + output stores
    on the ACT engine/ring, gate-muls alone on DVE - keeps each in-order
    sequencer free of cross-engine stalls.
  - Host only reorders rows (sharding) and un-permutes the output
    (unsharding); all arithmetic happens on device.

Measured on 8 axon-tunneled trn2 cores: ~147-156 us steady-state per pass
(in-kernel repetition slope), rel err 1.2e-4. fp32r stream floor for this
schedule is ~142 us; HBM floor ~90 us.
"""

import os
import numpy as np

_P = 128  # partitions / token tile


def _routing_layout(gmat, n_cores):
    """gmat: [T, NE] gates. Returns (T_core, slot_lists, tokmap).

    A "tile" is 128 token positions on every core (128*n_cores globally).
    Tokens grouped by nonzero-expert set; each group fills whole tiles
    ("pure", slots = the group's experts); the remainders are bin-packed
    into shared spill tiles whose slots are the union of the packed groups
    (zero gates make wrong-expert contributions exactly 0). Per-core
    padding is at most one tile's worth in the final bin.

    slot_lists: per tile, expert ids active in that tile (same on all cores).
    tokmap: [n_cores, T_core] global token index per position, -1 = pad.
    """
    from itertools import permutations

    T, NE = gmat.shape
    cap = n_cores * _P  # global tokens per tile
    mask = ((gmat > 0).astype(np.int32) * (1 << np.arange(NE, dtype=np.int32))).sum(
        axis=1
    )
    order = np.argsort(mask, kind="stable")
    uniq, starts = np.unique(mask[order], return_index=True)
    starts = list(starts) + [T]

    def experts_of(m):
        return frozenset(e for e in range(NE) if (int(m) >> e) & 1)

    tiles = []  # (expert_set, token_array of len <= cap)
    spills = []  # (expert_set, token_array)
    for i, m in enumerate(uniq):
        toks = order[starts[i] : starts[i + 1]]
        es = experts_of(m)
        n_pure = len(toks) // cap
        for j in range(n_pure):
            tiles.append((es, toks[j * cap : (j + 1) * cap]))
        rem = toks[n_pure * cap :]
        if len(rem):
            spills.append((es, rem))

    # order spill groups, fill bins of `cap` sequentially (groups may split
    # across adjacent bins); pick the order minimizing total slot cost
    def pack_cost(perm):
        bins = []
        fill, union = 0, set()
        unions = []
        for es, toks in perm:
            left = len(toks)
            union |= es
            while fill + left >= cap:
                take = cap - fill
                left -= take
                unions.append(union)
                union = set(es) if left else set()
                fill = 0
            fill += left
            if left == 0 and fill == 0 and not union:
                union = set()
        if fill or union:
            unions.append(union)
        return sum(16 * len(u) + 2 for u in unions), unions

    if spills:
        if len(spills) <= 7:
            cands = permutations(spills)
        else:
            cands = [sorted(spills, key=lambda s: -len(s[1]))]
        best_perm, best_cost = None, None
        for perm in cands:
            c, _ = pack_cost(list(perm))
            if best_cost is None or c < best_cost:
                best_cost, best_perm = c, list(perm)

        seq = np.concatenate([toks for _, toks in best_perm])
        bounds = np.cumsum([0] + [len(toks) for _, toks in best_perm])
        n_bins = -(-len(seq) // cap)
        for bi in range(n_bins):
            lo, hi = bi * cap, min((bi + 1) * cap, len(seq))
            u = set()
            for gi, (es, toks) in enumerate(best_perm):
                if bounds[gi] < hi and bounds[gi + 1] > lo:
                    u |= es
            tiles.append((frozenset(u), seq[lo:hi]))

    slot_lists = [sorted(es) for es, _ in tiles]
    T_core = _P * len(tiles)
    tokmap = np.full((n_cores, T_core), -1, dtype=np.int64)
    for ti, (_, toks) in enumerate(tiles):
        for c, ch in enumerate(np.array_split(toks, n_cores)):
            tokmap[c, ti * _P : ti * _P + len(ch)] = ch
    return T_core, slot_lists, tokmap


def _build_program(T_core, slot_lists, NE, D_in, D_out, reps=1):
    from contextlib import ExitStack

    import concourse.bacc as bacc
    import concourse.mybir as mybir
    from concourse.tile import TileContext

    f32 = mybir.dt.float32
    f32r = mybir.dt.float32r

    KT = D_in // _P
    n_tiles = T_core // _P
    n_slot = sum(len(s) for s in slot_lists)
    OH = D_out // 2  # psum half width (<=512 for 4-byte matmul)

    nc = bacc.Bacc(trn_type="TRN2", target_bir_lowering=False, debug=False)

    xT = nc.dram_tensor("xT", [D_in, T_core], f32, kind="ExternalInput").ap()
    wt = nc.dram_tensor("wt", [NE, D_in, D_out], f32r, kind="ExternalInput").ap()
    gbc = nc.dram_tensor("gbc", [_P, n_slot * _P], f32, kind="ExternalInput").ap()
    gT = nc.dram_tensor("gT", [_P, T_core], f32r, kind="ExternalInput").ap()
    bias = nc.dram_tensor("bias", [_P, D_out], f32r, kind="ExternalInput").ap()
    outp = nc.dram_tensor("outp", [T_core, D_out], f32, kind="ExternalOutput").ap()

    with TileContext(nc) as tc, ExitStack() as ctx:
        const = ctx.enter_context(tc.tile_pool(name="const", bufs=1))
        xt_pool = ctx.enter_context(tc.tile_pool(name="xt", bufs=2))
        xg_pool = ctx.enter_context(tc.tile_pool(name="xg", bufs=4))
        out_pool = ctx.enter_context(tc.tile_pool(name="out", bufs=3))
        ps_pool = ctx.enter_context(tc.tile_pool(name="ps", bufs=8, space="PSUM"))

        # startup loads spread across queues so W streams from t=0:
        # gbc on the SP ring (ahead of the xt chunks), gT/bias on the ACT
        # ring, W alone on SWDGE in 2MB chunks in first-use order
        # Everything streams through one HBM pipe; the SWDGE FIFO gives exact
        # control of delivery order, so issue strictly by first use:
        # gate-head -> first W chunk -> first x chunk -> rest of tile-0/1 W
        # -> gates/bias (needed at first chain tails) -> remaining gates ->
        # W of later experts.
        gbc_sb = const.tile([_P, n_slot, _P], f32, tag="gbc")
        gbc_r = gbc.rearrange("p (s t) -> p s t", t=_P)
        s_head = sum(len(s) for s in slot_lists[:2]) if n_tiles > 2 else n_slot
        nc.gpsimd.dma_start(gbc_sb[:, :s_head, :], gbc_r[:, :s_head, :])

        expert_order = []
        for slots in slot_lists:
            for e in slots:
                if e not in expert_order:
                    expert_order.append(e)
        for e in range(NE):
            if e not in expert_order:
                expert_order.append(e)

        w_sb = const.tile([_P, NE, KT, D_out], f32r, tag="w")

        def w_chunk(e, k0):
            wtr = wt[e].rearrange("(kk p) o -> p kk o", p=_P)
            nc.gpsimd.dma_start(w_sb[:, e, k0 : k0 + 4, :], wtr[:, k0 : k0 + 4, :])

        CH0 = min(2, n_tiles)  # first xt chunk, preloaded ahead of W
        xt_first = xt_pool.tile([_P, KT, 2 * _P], f32, tag="xt")
        nc.gpsimd.dma_start(
            xt_first[:, :, : CH0 * _P],
            xT[:, : CH0 * _P].rearrange("(kk p) t -> p kk t", p=_P),
        )
        e0 = expert_order[0]
        w_chunk(e0, 0)
        w_chunk(e0, 4)
        tile0_experts = expert_order[: max(1, len(slot_lists[0]))]
        for e in tile0_experts[1:]:
            w_chunk(e, 0)
            w_chunk(e, 4)

        gT_sb = const.tile([_P, T_core], f32r, tag="gT")
        nc.gpsimd.dma_start(gT_sb[:], gT)
        bias_sb = const.tile([_P, D_out], f32r, tag="bias")
        nc.gpsimd.dma_start(bias_sb[:], bias)
        if s_head < n_slot:
            nc.gpsimd.dma_start(gbc_sb[:, s_head:, :], gbc_r[:, s_head:, :])

        for e in expert_order:
            if e in tile0_experts:
                continue
            w_chunk(e, 0)
            w_chunk(e, 4)

        CH = 2  # token tiles per xt DMA chunk (bigger descriptors)

        def _one_pass(preloaded=None):
          slot_base = 0
          xt_ch = None
          for i in range(n_tiles):
            slots = slot_lists[i]
            if i % CH == 0:
                if i == 0 and preloaded is not None:
                    xt_ch = preloaded
                else:
                    nchunk = min(CH, n_tiles - i)
                    xt_ch = xt_pool.tile([_P, KT, CH * _P], f32, tag="xt")
                    nc.sync.dma_start(
                        xt_ch[:, :, : nchunk * _P],
                        xT[:, i * _P : (i + nchunk) * _P].rearrange(
                            "(kk p) t -> p kk t", p=_P
                        ),
                    )
            off = (i % CH) * _P
            xt = xt_ch[:, :, off : off + _P]
            xgs = []
            for j in range(len(slots)):
                xg = xg_pool.tile([_P, KT, _P], f32r, tag="xg")
                gb = (
                    gbc_sb[:, slot_base + j, :]
                    .unsqueeze(1)
                    .broadcast_to([_P, KT, _P])
                )
                nc.vector.tensor_mul(xg[:], xt, gb)
                xgs.append(xg)
            out_t = out_pool.tile([_P, D_out], f32, tag="out")
            for h in range(2):
                psum = ps_pool.tile([_P, OH], f32, tag="ps")
                first = True
                for j, e in enumerate(slots):
                    for kk in range(KT):
                        nc.tensor.matmul(
                            psum[:],
                            xgs[j][:, kk, :],
                            w_sb[:, e, kk, h * OH : (h + 1) * OH],
                            start=first,
                            stop=False,
                        )
                        first = False
                nc.tensor.matmul(
                    psum[:],
                    gT_sb[:, i * _P : (i + 1) * _P],
                    bias_sb[:, h * OH : (h + 1) * OH],
                    start=first,
                    stop=True,
                )
                nc.scalar.copy(out_t[:, h * OH : (h + 1) * OH], psum[:])
            nc.scalar.dma_start(outp[i * _P : (i + 1) * _P, :], out_t[:])
            slot_base += len(slots)

        for _rep in range(reps):
            _one_pass(preloaded=xt_first if _rep == 0 else None)

    nc.compile()
    return nc


def _prepare(input, gates, W, b, n_cores=8):
    """Host-side sharding. Returns (nc, in_maps, tokmap, out_shape)."""
    in_shape = input.shape
    D_in = in_shape[-1]
    NE, D_out = b.shape
    x = np.ascontiguousarray(input.reshape(-1, D_in), dtype=np.float32)
    g = np.ascontiguousarray(gates.reshape(-1, NE), dtype=np.float32)

    if os.environ.get("KERNEL_DENSE"):
        T = x.shape[0]
        T_core = T // n_cores
        slot_lists = [list(range(NE))] * (T_core // _P)
        tokmap = np.arange(T, dtype=np.int64).reshape(n_cores, T_core)
    else:
        T_core, slot_lists, tokmap = _routing_layout(g, n_cores)

    reps = int(os.environ.get("KERNEL_REPS", "1"))
    nc = _build_program(T_core, slot_lists, NE, D_in, D_out, reps=reps)

    wt = np.ascontiguousarray(np.transpose(W, (0, 2, 1)).astype(np.float32))
    b_np = np.ascontiguousarray(b.astype(np.float32))
    n_slot = sum(len(s) for s in slot_lists)

    in_maps = []
    for c in range(n_cores):
        idx = tokmap[c]
        valid = idx >= 0
        xc = np.zeros((T_core, D_in), np.float32)
        xc[valid] = x[idx[valid]]
        gc = np.zeros((T_core, NE), np.float32)
        gc[valid] = g[idx[valid]]

        gv = np.empty((n_slot, _P), np.float32)
        s = 0
        for i, slots in enumerate(slot_lists):
            for e in slots:
                gv[s] = gc[i * _P : (i + 1) * _P, e]
                s += 1
        gbc = np.ascontiguousarray(
            np.broadcast_to(gv.reshape(1, n_slot * _P), (_P, n_slot * _P))
        )
        gT128 = np.zeros((_P, T_core), np.float32)
        gT128[:NE] = gc.T
        bias128 = np.zeros((_P, D_out), np.float32)
        bias128[:NE] = b_np
        in_maps.append(
            {
                "xT": np.ascontiguousarray(xc.T),
                "wt": wt,
                "gbc": gbc,
                "gT": gT128,
                "bias": bias128,
            }
        )
    return nc, in_maps, tokmap, in_shape[:-1] + (D_out,)


def _gather(results, tokmap, out_shape, D_out):
    T = int(np.prod(out_shape[:-1]))
    out = np.empty((T, D_out), np.float32)
    for c in range(tokmap.shape[0]):
        idx = tokmap[c]
        valid = idx >= 0
        out[idx[valid]] = results[c]["outp"][valid]
    return out.reshape(out_shape)


def kernel(input, gates, W, b):
    from concourse.bass_utils import run_bass_kernel_spmd

    n_cores = 8
    nc, in_maps, tokmap, out_shape = _prepare(input, gates, W, b, n_cores)
    res = run_bass_kernel_spmd(nc, in_maps, core_ids=list(range(n_cores))).results
    return _gather(res, tokmap, out_shape, b.shape[1])


# revision 26
# speedup vs baseline: 1.0062x; 1.0062x over previous
"""Trainium2 Bass kernel for nn_Affine_83811991814656 (MoE routing).

Math (reference):
    out[t] = sum_e gates[t, e] * (x[t] @ W[e].T + b[e])

Strategy:
  - Tokens (B*S = 16384) are data-parallel across 8 NeuronCores. SPMD: one
    program, per-core data, so the tile -> experts map must be identical on
    every core.
  - Gates are top-2-of-4 sparse. Host groups tokens by nonzero-expert set
    (6 pair groups typically). Each group fills whole 128-token-per-core
    tiles ("pure" tiles, 2 expert slots); the <1-tile remainders are
    bin-packed into ~3 shared spill tiles whose slots are the union of the
    packed groups (zero gates annihilate wrong-expert contributions), with
    the packing order brute-forced to minimize slot count. Zero padding in
    the typical case: 16 tiles, ~35 slots vs 64 for a dense kernel.
  - Per tile the gate scaling is folded into the matmul lhsT (xg = xT * g
    broadcast along partitions, on DVE), so all expert matmuls AND the
    bias matmul (gates^T @ b, zero-padded to K=128 - a K=4 fp32r matmul
    measures ~3x slower than K=128) accumulate into one PSUM bank per
    output half. fp32r matmuls: full fp32 storage, PE rounds to ~13
    mantissa bits (rel err ~1.5e-4), 4x faster than strict fp32, and the
    ~512-row stream dominates so per-matmul cost is ~240 ns.
  - W (all 4 experts, transposed to [d, o]) stays resident in SBUF (16 MB).
  - DMA spread over the three queues: W/gbc/gates on the SWDGE (gpsimd)
    queue, x loads on the SP HWDGE ring, psum->SBUF copies + output stores
    on the ACT engine/ring, gate-muls alone on DVE - keeps each in-order
    sequencer free of cross-engine stalls.
  - Host only reorders rows (sharding) and un-permutes the output
    (unsharding); all arithmetic happens on device.

Measured on 8 axon-tunneled trn2 cores: ~147-156 us steady-state per pass
(in-kernel repetition slope), rel err 1.2e-4. fp32r stream floor for this
schedule is ~142 us; HBM floor ~90 us.
"""

import os
import numpy as np

_P = 128  # partitions / token tile


def _routing_layout(gmat, n_cores):
    """gmat: [T, NE] gates. Returns (T_core, slot_lists, tokmap).

    A "tile" is 128 token positions on every core (128*n_cores globally).
    Tokens grouped by nonzero-expert set; each group fills whole tiles
    ("pure", slots = the group's experts); the remainders are bin-packed
    into shared spill tiles whose slots are the union of the packed groups
    (zero gates make wrong-expert contributions exactly 0). Per-core
    padding is at most one tile's worth in the final bin.

    slot_lists: per tile, expert ids active in that tile (same on all cores).
    tokmap: [n_cores, T_core] global token index per position, -1 = pad.
    """
    from itertools import permutations

    T, NE = gmat.shape
    cap = n_cores * _P  # global tokens per tile
    mask = ((gmat > 0).astype(np.int32) * (1 << np.arange(NE, dtype=np.int32))).sum(
        axis=1
    )
    order = np.argsort(mask, kind="stable")
    uniq, starts = np.unique(mask[order], return_index=True)
    starts = list(starts) + [T]

    def experts_of(m):
        return frozenset(e for e in range(NE) if (int(m) >> e) & 1)

    tiles = []  # (expert_set, token_array of len <= cap)
    spills = []  # (expert_set, token_array)
    for i, m in enumerate(uniq):
        toks = order[starts[i] : starts[i + 1]]
        es = experts_of(m)
        n_pure = len(toks) // cap
        for j in range(n_pure):
            tiles.append((es, toks[j * cap : (j + 1) * cap]))
        rem = toks[n_pure * cap :]
        if len(rem):
            spills.append((es, rem))

    # order spill groups, fill bins of `cap` sequentially (groups may split
    # across adjacent bins); pick the order minimizing total slot cost
    def pack_cost(perm):
        bins = []
        fill, union = 0, set()
        unions = []
        for es, toks in perm:
            left = len(toks)
            union |= es
            while fill + left >= cap:
                take = cap - fill
                left -= take
                unions.append(union)
                union = set(es) if left else set()
                fill = 0
            fill += left
            if left == 0 and fill == 0 and not union:
                union = set()
        if fill or union:
            unions.append(union)
        return sum(16 * len(u) + 2 for u in unions), unions

    if spills:
        if len(spills) <= 7:
            cands = permutations(spills)
        else:
            cands = [sorted(spills, key=lambda s: -len(s[1]))]
        best_perm, best_cost = None, None
        for perm in cands:
            c, _ = pack_cost(list(perm))
            if best_cost is None or c < best_cost:
                best_cost, best_perm = c, list(perm)

        seq = np.concatenate([toks for _, toks in best_perm])
        bounds = np.cumsum([0] + [len(toks) for _, toks in best_perm])
        n_bins = -(-len(seq) // cap)
        for bi in range(n_bins):
            lo, hi = bi * cap, min((bi + 1) * cap, len(seq))
            u = set()
            for gi, (es, toks) in enumerate(best_perm):
                if bounds[gi] < hi and bounds[gi + 1] > lo:
                    u |= es
            tiles.append((frozenset(u), seq[lo:hi]))

    slot_lists = [sorted(es) for es, _ in tiles]
    T_core = _P * len(tiles)
    tokmap = np.full((n_cores, T_core), -1, dtype=np.int64)
    for ti, (_, toks) in enumerate(tiles):
        for c, ch in enumerate(np.array_split(toks, n_cores)):
            tokmap[c, ti * _P : ti * _P + len(ch)] = ch
    return T_core, slot_lists, tokmap


def _build_program(T_core, slot_lists, NE, D_in, D_out, reps=1):
    from contextlib import ExitStack

    import concourse.bacc as bacc
    import concourse.mybir as mybir
    from concourse.tile import TileContext

    f32 = mybir.dt.float32
    f32r = mybir.dt.float32r

    KT = D_in // _P
    n_tiles = T_core // _P
    n_slot = sum(len(s) for s in slot_lists)
    OH = D_out // 2  # psum half width (<=512 for 4-byte matmul)

    nc = bacc.Bacc(trn_type="TRN2", target_bir_lowering=False, debug=False)

    xT = nc.dram_tensor("xT", [D_in, T_core], f32, kind="ExternalInput").ap()
    wt = nc.dram_tensor("wt", [NE, D_in, D_out], f32r, kind="ExternalInput").ap()
    gbc = nc.dram_tensor("gbc", [_P, n_slot * _P], f32, kind="ExternalInput").ap()
    gT = nc.dram_tensor("gT", [_P, T_core], f32r, kind="ExternalInput").ap()
    bias = nc.dram_tensor("bias", [_P, D_out], f32r, kind="ExternalInput").ap()
    outp = nc.dram_tensor("outp", [T_core, D_out], f32, kind="ExternalOutput").ap()

    with TileContext(nc) as tc, ExitStack() as ctx:
        const = ctx.enter_context(tc.tile_pool(name="const", bufs=1))
        xt_pool = ctx.enter_context(tc.tile_pool(name="xt", bufs=2))
        xg_pool = ctx.enter_context(tc.tile_pool(name="xg", bufs=4))
        out_pool = ctx.enter_context(tc.tile_pool(name="out", bufs=3))
        ps_pool = ctx.enter_context(tc.tile_pool(name="ps", bufs=8, space="PSUM"))

        # startup loads spread across queues so W streams from t=0:
        # gbc on the SP ring (ahead of the xt chunks), gT/bias on the ACT
        # ring, W alone on SWDGE in 2MB chunks in first-use order
        # Everything streams through one HBM pipe; the SWDGE FIFO gives exact
        # control of delivery order, so issue strictly by first use:
        # gate-head -> first W chunk -> first x chunk -> rest of tile-0/1 W
        # -> gates/bias (needed at first chain tails) -> remaining gates ->
        # W of later experts.
        gbc_sb = const.tile([_P, n_slot, _P], f32, tag="gbc")
        gbc_r = gbc.rearrange("p (s t) -> p s t", t=_P)
        s_head = sum(len(s) for s in slot_lists[:2]) if n_tiles > 2 else n_slot
        nc.gpsimd.dma_start(gbc_sb[:, :s_head, :], gbc_r[:, :s_head, :])

        expert_order = []
        for slots in slot_lists:
            for e in slots:
                if e not in expert_order:
                    expert_order.append(e)
        for e in range(NE):
            if e not in expert_order:
                expert_order.append(e)

        w_sb = const.tile([_P, NE, KT, D_out], f32r, tag="w")

        def w_chunk(e, k0):
            wtr = wt[e].rearrange("(kk p) o -> p kk o", p=_P)
            nc.gpsimd.dma_start(w_sb[:, e, k0 : k0 + 4, :], wtr[:, k0 : k0 + 4, :])

        CH0 = min(2, n_tiles)  # first xt chunk, preloaded ahead of W
        xt_first = xt_pool.tile([_P, KT, 2 * _P], f32, tag="xt")
        nc.gpsimd.dma_start(
            xt_first[:, :, : CH0 * _P],
            xT[:, : CH0 * _P].rearrange("(kk p) t -> p kk t", p=_P),
        )
        e0 = expert_order[0]
        w_chunk(e0, 0)
        w_chunk(e0, 4)
        tile0_experts = expert_order[: max(1, len(slot_lists[0]))]
        for e in tile0_experts[1:]:
            w_chunk(e, 0)
            w_chunk(e, 4)

        gT_sb = const.tile([_P, T_core], f32r, tag="gT")
        nc.gpsimd.dma_start(gT_sb[:], gT)
        bias_sb = const.tile([_P, D_out], f32r, tag="bias")
        nc.gpsimd.dma_start(bias_sb[:], bias)
        if s_head < n_slot:
            nc.gpsimd.dma_start(gbc_sb[:, s_head:, :], gbc_r[:, s_head:, :])

        for e in expert_order:
            if e in tile0_experts:
                continue
            w_chunk(e, 0)
            w_chunk(e, 4)

        CH = 2  # token tiles per xt DMA chunk (bigger descriptors)

        def _one_pass(preloaded=None):
          slot_base = 0
          xt_ch = None
          for i in range(n_tiles):
            slots = slot_lists[i]
            if i % CH == 0:
                if i == 0 and preloaded is not None:
                    xt_ch = preloaded
                else:
                    nchunk = min(CH, n_tiles - i)
                    xt_ch = xt_pool.tile([_P, KT, CH * _P], f32, tag="xt")
                    nc.sync.dma_start(
                        xt_ch[:, :, : nchunk * _P],
                        xT[:, i * _P : (i + nchunk) * _P].rearrange(
                            "(kk p) t -> p kk t", p=_P
                        ),
                    )
            off = (i % CH) * _P
            xt = xt_ch[:, :, off : off + _P]
            xgs = []
            for j in range(len(slots)):
                xg = xg_pool.tile([_P, KT, _P], f32r, tag="xg")
                gb = (
                    gbc_sb[:, slot_base + j, :]
                    .unsqueeze(1)
                    .broadcast_to([_P, KT, _P])
                )
                nc.vector.tensor_mul(xg[:], xt, gb)
                xgs.append(xg)
            out_t = out_pool.tile([_P, D_out], f32, tag="out")
            # two interleaved psum chains (output halves) per tile: each
            # (slot, k) lhsT is loaded once and the second matmul reuses it
            # via ldweights=False (verified exact on HW, ~17ns/MM saved)
            psA = ps_pool.tile([_P, OH], f32, tag="ps")
            psB = ps_pool.tile([_P, OH], f32, tag="ps")
            first = True
            for j, e in enumerate(slots):
                for kk in range(KT):
                    nc.tensor.matmul(
                        psA[:],
                        xgs[j][:, kk, :],
                        w_sb[:, e, kk, :OH],
                        start=first,
                        stop=False,
                        skip_group_check=True,
                    )
                    m2 = nc.tensor.matmul(
                        psB[:],
                        xgs[j][:, kk, :],
                        w_sb[:, e, kk, OH:],
                        start=first,
                        stop=False,
                        skip_group_check=True,
                    )
                    m2.ins.ldweights = False
                    first = False
            nc.tensor.matmul(
                psA[:],
                gT_sb[:, i * _P : (i + 1) * _P],
                bias_sb[:, :OH],
                start=first,
                stop=True,
                skip_group_check=True,
            )
            m2 = nc.tensor.matmul(
                psB[:],
                gT_sb[:, i * _P : (i + 1) * _P],
                bias_sb[:, OH:],
                start=first,
                stop=True,
                skip_group_check=True,
            )
            m2.ins.ldweights = False
            nc.scalar.copy(out_t[:, :OH], psA[:])
            nc.scalar.copy(out_t[:, OH:], psB[:])
            nc.scalar.dma_start(outp[i * _P : (i + 1) * _P, :], out_t[:])
            slot_base += len(slots)

        for _rep in range(reps):
            _one_pass(preloaded=xt_first if _rep == 0 else None)

    nc.compile()
    return nc


def _prepare(input, gates, W, b, n_cores=8):
    """Host-side sharding. Returns (nc, in_maps, tokmap, out_shape)."""
    in_shape = input.shape
    D_in = in_shape[-1]
    NE, D_out = b.shape
    x = np.ascontiguousarray(input.reshape(-1, D_in), dtype=np.float32)
    g = np.ascontiguousarray(gates.reshape(-1, NE), dtype=np.float32)

    if os.environ.get("KERNEL_DENSE"):
        T = x.shape[0]
        T_core = T // n_cores
        slot_lists = [list(range(NE))] * (T_core // _P)
        tokmap = np.arange(T, dtype=np.int64).reshape(n_cores, T_core)
    else:
        T_core, slot_lists, tokmap = _routing_layout(g, n_cores)

    reps = int(os.environ.get("KERNEL_REPS", "1"))
    nc = _build_program(T_core, slot_lists, NE, D_in, D_out, reps=reps)

    wt = np.ascontiguousarray(np.transpose(W, (0, 2, 1)).astype(np.float32))
    b_np = np.ascontiguousarray(b.astype(np.float32))
    n_slot = sum(len(s) for s in slot_lists)

    in_maps = []
    for c in range(n_cores):
        idx = tokmap[c]
        valid = idx >= 0
        xc = np.zeros((T_core, D_in), np.float32)
        xc[valid] = x[idx[valid]]
        gc = np.zeros((T_core, NE), np.float32)
        gc[valid] = g[idx[valid]]

        gv = np.empty((n_slot, _P), np.float32)
        s = 0
        for i, slots in enumerate(slot_lists):
            for e in slots:
                gv[s] = gc[i * _P : (i + 1) * _P, e]
                s += 1
        gbc = np.ascontiguousarray(
            np.broadcast_to(gv.reshape(1, n_slot * _P), (_P, n_slot * _P))
        )
        gT128 = np.zeros((_P, T_core), np.float32)
        gT128[:NE] = gc.T
        bias128 = np.zeros((_P, D_out), np.float32)
        bias128[:NE] = b_np
        in_maps.append(
            {
                "xT": np.ascontiguousarray(xc.T),
                "wt": wt,
                "gbc": gbc,
                "gT": gT128,
                "bias": bias128,
            }
        )
    return nc, in_maps, tokmap, in_shape[:-1] + (D_out,)


def _gather(results, tokmap, out_shape, D_out):
    T = int(np.prod(out_shape[:-1]))
    out = np.empty((T, D_out), np.float32)
    for c in range(tokmap.shape[0]):
        idx = tokmap[c]
        valid = idx >= 0
        out[idx[valid]] = results[c]["outp"][valid]
    return out.reshape(out_shape)


def kernel(input, gates, W, b):
    from concourse.bass_utils import run_bass_kernel_spmd

    n_cores = 8
    nc, in_maps, tokmap, out_shape = _prepare(input, gates, W, b, n_cores)
    res = run_bass_kernel_spmd(nc, in_maps, core_ids=list(range(n_cores))).results
    return _gather(res, tokmap, out_shape, b.shape[1])
